# revision 32
# baseline (speedup 1.0000x reference)
"""Trainium2 Bass kernel for the QRNN-style recommender model.

Model (per batch row b):
  emb = item_emb[seq]                          # [T=16, D=256]
  conv_out[l,t,c] = sum_{m<=l} emb[t-m] @ W[l,m,c,:] + conv_b[l,c]   (L=16 causal convs)
  f = sigmoid(relu(conv_out)) = max(sigmoid(conv_out), 0.5)
  h = fo-pool chain applied 3x over t (QRNN), x0 = emb
  o = sum over (l, t) of h                     # [D]
  z = [o, user_emb[user]] @ fc1_w.T + fc1_b    # [D]
  res[n] = W2[item[n]] . z + b2[item[n]]       # [N_TGT=32]

Sharding: data-parallel over batch B=512 across 8 cores (64 rows each);
all parameters/tables replicated; embedding gathers via indirect DMA.

Per-core layout (v2 — unrolled fo-pool, no tensor_tensor_scan):
  embTb[kc][d(128), t(16), b(64)]  bf16 transposed gathered emb (conv rhs)
  X0b[d(128), t, cc(2), b]         fp16 emb for the fo-pool x0
  conv: per (l, cc) one psum tile [c(128), bank(2), t8, b64] (t-split
        banks, col = 64*t + b), accumulated over (m, kc) bf16 matmuls
  gates: one ACT sigmoid per (l, cc) -> F[d, t, cc, l8, b] fp16;
        DVE tensor_scalar max(., 0.5) clamp in place
  fo-pool: 2 rounds of 8 l's; per round 3 chains x 16 unrolled steps of
        fp16 DVE tensor_tensor (d = x - h; e = f*d; h = h + e), each op
        [128, 2*8*64] at DVE 2x rate; x of chain 0 broadcasts X0b over l
  o: in-place binary-tree adds over t then l, accumulated into oacc f32
  head: fc1 via PE, per-row dot with gathered W2 rows via DVE mul +
        ones-vector PE partition-reduction.
"""
import os
import numpy as np

import concourse.bass as bass
import concourse.mybir as mybir
import concourse.tile as tile
from concourse import bacc
from concourse.masks import make_identity

F32 = mybir.dt.float32
BF16 = mybir.dt.bfloat16
F16 = mybir.dt.float16
F8 = mybir.dt.float8e4
I32 = mybir.dt.int32
EMB_SCALE = 128.0          # emb fp8 pre-scale
W_SCALE = 32.0             # conv weight fp8 pre-scale
Z_DESCALE = 1.0 / (EMB_SCALE * W_SCALE)
AF = mybir.ActivationFunctionType
ALU = mybir.AluOpType

# model dims (hardcoded per problem spec)
N_CORES = 8
B = 512
BC = B // N_CORES          # 64 rows per core
T = 16
L = 16
L8 = 8                     # l's per round
D = 256
N_TGT = 32
N_ITEMS = 200000
N_USERS = 100000
N_L = 3                    # fo-pool chain depth
TRI = [l * (l + 1) // 2 for l in range(L + 1)]  # block offsets for (l, m<=l)


def _build_kernel(nc, tc):
    seqT = nc.dram_tensor("seqT", [128, 8], I32, kind="ExternalInput").ap()
    itemT = nc.dram_tensor("itemT", [128, 16], I32, kind="ExternalInput").ap()
    useri = nc.dram_tensor("useri", [BC], I32, kind="ExternalInput").ap()
    item_emb = nc.dram_tensor("item_emb", [N_ITEMS, D], F32, kind="ExternalInput").ap()
    user_emb = nc.dram_tensor("user_emb", [N_USERS, D], F32, kind="ExternalInput").ap()
    w2tab = nc.dram_tensor("w2tab", [N_ITEMS, D], F32, kind="ExternalInput").ap()
    wt = nc.dram_tensor("wt", [TRI[L], D, D], F8, kind="ExternalInput").ap()
    convb = nc.dram_tensor("convb", [128, 2, L], F32, kind="ExternalInput").ap()
    fc1wt = nc.dram_tensor("fc1wt", [2 * D, D], F32, kind="ExternalInput").ap()
    fc1b = nc.dram_tensor("fc1b", [128, 2], F32, kind="ExternalInput").ap()
    res = nc.dram_tensor("res", [BC, N_TGT], F32, kind="ExternalOutput").ap()

    import contextlib
    ctx = contextlib.ExitStack()
    with ctx:
        perm = ctx.enter_context(tc.tile_pool(name="perm", bufs=1))
        idxp = ctx.enter_context(tc.tile_pool(name="idxp", bufs=2))
        gath = ctx.enter_context(tc.tile_pool(name="gath", bufs=4))
        wpool = ctx.enter_context(tc.tile_pool(name="wpool", bufs=12))
        st = ctx.enter_context(tc.tile_pool(name="st", bufs=2))
        small = ctx.enter_context(tc.tile_pool(name="small", bufs=2))
        cps = ctx.enter_context(tc.tile_pool(name="cps", bufs=3, space="PSUM"))
        tps = ctx.enter_context(tc.tile_pool(name="tps", bufs=2, space="PSUM"))

        # all seq indices in one DMA so the 8 gathers issue back-to-back
        seqi = perm.tile([128, 8], I32, tag="seqi")
        nc.sync.dma_start(seqi[:], seqT[:])

        cb = perm.tile([128, 2, L], F32, tag="cb")
        nc.sync.dma_start(cb[:], convb[:])
        f1w = perm.tile([128, 4, D], F32, tag="f1w")
        nc.sync.dma_start(f1w[:], fc1wt.rearrange("(kc k) c -> k kc c", k=128))
        f1b = perm.tile([128, 2], F32, tag="f1b")
        nc.sync.dma_start(f1b[:], fc1b[:])

        # ---- phase A: gather seq embeddings -> embT8 (fp8, scaled), X0b (fp16)
        embT8 = perm.tile([128, 2, T, BC], F8, tag="embt8")
        X0b = perm.tile([128, T, 2, BC], F16, tag="x0b")
        gts = []
        for c in range(8):
            gt = gath.tile([128, D], F32, tag="embg", bufs=8, name=f"gt{c}")
            nc.gpsimd.indirect_dma_start(
                out=gt[:], out_offset=None, in_=item_emb[:],
                in_offset=bass.IndirectOffsetOnAxis(ap=seqi[:, c:c + 1], axis=0))
            gts.append(gt)
        ident = perm.tile([128, 128], F32, tag="ident")
        make_identity(nc, ident)
        for c in range(8):
            gt = gts[c]
            for kc in (0, 1):
                tp = tps.tile([128, 128], F32, tag="tp")
                nc.tensor.transpose(tp[:], gt[:, kc * 128:(kc + 1) * 128], ident[:])
                tpv = tp[:].rearrange("p (b t) -> p t b", b=8)
                nc.scalar.mul(embT8[:, kc, :, 8 * c:8 * (c + 1)], tpv, EMB_SCALE)
                nc.vector.tensor_copy(out=X0b[:, :, kc, 8 * c:8 * (c + 1)], in_=tpv)

        # ---- big fp16 state tiles (round 0 split into two 4-l sub-rounds
        # so DVE steps start ~40us earlier; round 1 is one efficient 8-l round)
        F0a = perm.tile([128, T, 2, 4, BC], F16, tag="f0a")
        F0b = perm.tile([128, T, 2, 4, BC], F16, tag="f0b")
        F1 = perm.tile([128, T, 2, L8, BC], F16, tag="f1")
        HA = perm.tile([128, T, 2, L8, BC], F16, tag="ha")
        HB = perm.tile([128, T, 2, L8, BC], F16, tag="hb")
        oacc = [perm.tile([128, BC], F32, tag=f"oacc{cc}", name=f"oacc{cc}")
                for cc in (0, 1)]

        def conv_gates(l_lo, l_hi, Fti):
            """conv + sigmoid gates for l in [l_lo, l_hi) -> Fti."""
            DR = mybir.MatmulPerfMode.DoubleRow
            for gl in range(l_lo, l_hi):
                li = gl - l_lo
                m0max = min(gl, 7)
                # both cc psums live so each tap's weight is consumed immediately
                ps = [cps.tile([128, 2, 8, BC], F32, tag="cps", name=f"ps{cc}")
                      for cc in (0, 1)]
                for m in range(gl + 1):
                    w_t = wpool.tile([128, 2, D], F8, tag="wt")
                    nc.sync.dma_start(
                        w_t[:], wt[TRI[gl] + m].rearrange("(kc k) c -> k kc c", k=128))
                    for cc in (0, 1):
                        lhs = w_t[:, :, cc * 128:(cc + 1) * 128]
                        # bank 0: t in [0, 8)
                        if m <= 7:
                            nc.tensor.matmul(
                                ps[cc][:, 0, m:8, :], lhsT=lhs,
                                rhs=embT8[:, :, 0:8 - m, :],
                                start=(m == 0), stop=(m == m0max), perf_mode=DR)
                        # bank 1: t in [8, 16)
                        if m < 8:
                            out_ap = ps[cc][:, 1, :, :]
                            rhs = embT8[:, :, 8 - m:16 - m, :]
                        else:
                            out_ap = ps[cc][:, 1, m - 8:8, :]
                            rhs = embT8[:, :, 0:16 - m, :]
                        nc.tensor.matmul(
                            out_ap, lhsT=lhs, rhs=rhs,
                            start=(m == 0), stop=(m == gl), perf_mode=DR)
                for cc in (0, 1):
                    # f' = sigmoid(z); the relu fold (max with 0.5) happens as
                    # one whole-tile DVE clamp at the head of the fo-pool round
                    fsl = Fti[:, :, cc, li, :]
                    nc.scalar.activation(
                        fsl, ps[cc][:].rearrange("p bk t b -> p (bk t) b"),
                        AF.Sigmoid, bias=cb[:, cc, gl:gl + 1], scale=Z_DESCALE)

        def fopool(F, nl, rname):
            """triple fo-pool over t for gate tile F covering nl l's;
            returns O = sum over (t, l) as [128, 2, nl->1, BC]."""
            # f = sigmoid(relu(z)) = max(sigmoid(z), 0.5)
            nc.vector.tensor_scalar_max(out=F[:], in0=F[:], scalar1=0.5)
            ha = HA[:, :, :, 0:nl, :]
            hb = HB[:, :, :, 0:nl, :]
            # chains 1, 2 keep full h sequences (next chain's x)
            for xsrc, hout in ((None, ha), (ha, hb)):
                for t in range(T):
                    if xsrc is None:
                        xt = X0b[:, t, :, None, :].to_broadcast((128, 2, nl, BC))
                    else:
                        xt = xsrc[:, t]
                    ft = F[:, t]
                    if t == 0:
                        # h_0 = f_0 * x_0
                        nc.vector.tensor_tensor(
                            out=hout[:, 0], in0=ft, in1=xt, op=ALU.mult)
                    else:
                        # h_t = h_{t-1} + f_t * (x_t - h_{t-1})
                        d = st.tile([128, 2, nl, BC], F16, tag="std")
                        nc.vector.tensor_tensor(
                            out=d[:], in0=xt, in1=hout[:, t - 1], op=ALU.subtract)
                        e = st.tile([128, 2, nl, BC], F16, tag="ste")
                        nc.vector.tensor_tensor(
                            out=e[:], in0=ft, in1=d[:], op=ALU.mult)
                        nc.vector.tensor_tensor(
                            out=hout[:, t], in0=hout[:, t - 1], in1=e[:], op=ALU.add)
            # chain 3 with on-the-fly O accumulation
            O = perm.tile([128, 2, nl, BC], F16, tag=f"otile{rname}",
                          name=f"otile{rname}")
            h3p = None
            for t in range(T):
                xt = hb[:, t]
                ft = F[:, t]
                h3 = st.tile([128, 2, nl, BC], F16, tag="h3")
                if t == 0:
                    nc.vector.tensor_tensor(out=h3[:], in0=ft, in1=xt, op=ALU.mult)
                else:
                    d = st.tile([128, 2, nl, BC], F16, tag="std")
                    nc.vector.tensor_tensor(
                        out=d[:], in0=xt, in1=h3p[:], op=ALU.subtract)
                    e = st.tile([128, 2, nl, BC], F16, tag="ste")
                    nc.vector.tensor_tensor(out=e[:], in0=ft, in1=d[:], op=ALU.mult)
                    nc.vector.tensor_tensor(out=h3[:], in0=h3p[:], in1=e[:], op=ALU.add)
                    if t == 1:
                        nc.vector.tensor_tensor(out=O[:], in0=h3p[:], in1=h3[:], op=ALU.add)
                    else:
                        nc.vector.tensor_tensor(out=O[:], in0=O[:], in1=h3[:], op=ALU.add)
                h3p = h3
            # sum over l (in-place tree on O)
            wl = nl
            while wl > 1:
                wl //= 2
                nc.vector.tensor_tensor(
                    out=O[:, :, 0:wl], in0=O[:, :, 0:wl],
                    in1=O[:, :, wl:2 * wl], op=ALU.add)
            return O

        conv_gates(0, 4, F0a)
        conv_gates(4, 8, F0b)
        with tc.tile_wait_until(1.0):
            Oa = fopool(F0a, 4, "a")
            Ob = fopool(F0b, 4, "b")
        conv_gates(8, 16, F1)

        # ---- user embedding + W2 gathers/transposes: emitted here (unpinned)
        # so PE/ACT finish them during the fo-pool rounds, keeping the tail short
        uidx = idxp.tile([BC, 1], I32, tag="uidx")
        nc.sync.dma_start(uidx[:], useri[:, None])
        ug = gath.tile([BC, D], F32, tag="ug", bufs=1)
        nc.gpsimd.indirect_dma_start(
            out=ug[:], out_offset=None, in_=user_emb[:],
            in_offset=bass.IndirectOffsetOnAxis(ap=uidx[:, :1], axis=0))
        catT = [oacc[0], oacc[1]]
        for kc in (0, 1):
            tp = tps.tile([128, 128], F32, tag="tp")
            nc.tensor.transpose(tp[:, :BC], ug[:, kc * 128:(kc + 1) * 128], ident[:BC, :BC])
            ut = small.tile([128, BC], F32, tag=f"ut{kc}")
            nc.any.tensor_copy(ut[:], tp[:, :BC])
            catT.append(ut)

        # W2 row gathers -> w2t[kc] = [128, 2048] (c on partitions, (b,n) free)
        w2t = [perm.tile([128, BC * N_TGT], F32, tag=f"w2t{kc}", name=f"w2t{kc}")
               for kc in (0, 1)]
        itemi = perm.tile([128, 16], I32, tag="itemi")
        nc.sync.dma_start(itemi[:], itemT[:])
        for ch in range(16):
            wg = gath.tile([128, D], F32, tag="w2g")
            nc.gpsimd.indirect_dma_start(
                out=wg[:], out_offset=None, in_=w2tab[:],
                in_offset=bass.IndirectOffsetOnAxis(ap=itemi[:, ch:ch + 1], axis=0))
            for kc in (0, 1):
                tp = tps.tile([128, 128], F32, tag="tp")
                nc.tensor.transpose(tp[:], wg[:, kc * 128:(kc + 1) * 128], ident[:])
                nc.scalar.copy(w2t[kc][:, 128 * ch:128 * (ch + 1)], tp[:])

        with tc.tile_wait_until(2.5):
            Oc = fopool(F1, L8, "c")
        tail_pin = tc.tile_wait_until(3.0)
        tail_pin.__enter__()
        for cc in (0, 1):
            nc.scalar.copy(oacc[cc][:], Oa[:, cc, 0, :])
            for Ox in (Ob, Oc):
                stmp = small.tile([128, BC], F32, tag="stmp")
                nc.scalar.copy(stmp[:], Ox[:, cc, 0, :])
                nc.vector.tensor_tensor(
                    out=oacc[cc][:], in0=oacc[cc][:], in1=stmp[:], op=ALU.add)

        # ---- head: z^T = fc1_w @ cat^T + b  -> [zc(2 chunks of 128), b(64)]
        zT = []
        for cc in (0, 1):
            zp = tps.tile([128, BC], F32, tag="tp")
            for kcc in range(4):
                nc.tensor.matmul(
                    zp[:], lhsT=f1w[:, kcc, cc * 128:(cc + 1) * 128],
                    rhs=catT[kcc][:], start=(kcc == 0), stop=(kcc == 3))
            zt = small.tile([128, BC], F32, tag=f"zt{cc}")
            nc.scalar.activation(zt[:], zp[:], AF.Identity, bias=f1b[:, cc:cc + 1])
            zT.append(zt)

        # res[b,n] = sum_c w2t[c,(b,n)] * z[c,b]  (mul + ones-matmul partition sum)
        for kc in (0, 1):
            nc.vector.tensor_tensor(
                out=w2t[kc][:].rearrange("p (b n) -> p b n", n=N_TGT),
                in0=w2t[kc][:].rearrange("p (b n) -> p b n", n=N_TGT),
                in1=zT[kc][:, :, None].to_broadcast((128, BC, N_TGT)),
                op=ALU.mult)
        ones = small.tile([128, 1], F32, tag="ones")
        nc.vector.memset(ones[:], 1.0)
        res_sb = small.tile([1, BC * N_TGT], F32, tag="ressb", bufs=1)
        for j in range(4):
            rj = tps.tile([1, 512], F32, tag="tp")
            for kc in (0, 1):
                nc.tensor.matmul(rj[:], lhsT=ones[:],
                                 rhs=w2t[kc][:, 512 * j:512 * (j + 1)],
                                 start=(kc == 0), stop=(kc == 1))
            nc.any.tensor_copy(res_sb[:, 512 * j:512 * (j + 1)], rj[:])
        nc.sync.dma_start(res.rearrange("b n -> (b n)")[None, :], res_sb[:])
        tail_pin.__exit__(None, None, None)


_CACHED_NC = None


def build_nc():
    global _CACHED_NC
    if _CACHED_NC is not None:
        return _CACHED_NC
    nc = bacc.Bacc("TRN2", debug=False, enable_asserts=False)
    with tile.TileContext(nc) as tc:
        _build_kernel(nc, tc)
    nc.compile()
    _CACHED_NC = nc
    return nc


def make_in_maps(seq_var, user_var, item_var, item_emb, user_emb, conv_w,
                 conv_b, fc1_w, fc1_b, W2, b2):
    seq_var = np.asarray(seq_var).astype(np.int32)
    user_var = np.asarray(user_var).astype(np.int32)
    item_var = np.asarray(item_var).astype(np.int32)
    item_emb = np.ascontiguousarray(np.asarray(item_emb, dtype=np.float32))
    user_emb = np.ascontiguousarray(np.asarray(user_emb, dtype=np.float32))
    W2 = np.ascontiguousarray(np.asarray(W2, dtype=np.float32))
    conv_w = np.asarray(conv_w, dtype=np.float32)
    conv_b = np.ascontiguousarray(np.asarray(conv_b, dtype=np.float32))
    fc1_w = np.asarray(fc1_w, dtype=np.float32)
    fc1_b = np.ascontiguousarray(np.asarray(fc1_b, dtype=np.float32))

    # pack conv weights: block (l, m<=l) at TRI[l]+m = conv_w[l, m].T ([d, c]),
    # fp8 e4m3 scaled by W_SCALE (emb scaled by EMB_SCALE; ACT rescales psum)
    import ml_dtypes
    wt_pack = np.empty((TRI[L], D, D), ml_dtypes.float8_e4m3)
    for l in range(L):
        for m in range(l + 1):
            wt_pack[TRI[l] + m] = (conv_w[l, m].T * W_SCALE).astype(
                ml_dtypes.float8_e4m3)
    fc1wt = np.ascontiguousarray(fc1_w.T)
    # convb_pack[c, cc, l] = conv_b[l, cc*128 + c];  fc1b_pack[c, cc] = fc1_b[cc*128+c]
    convb_pack = np.ascontiguousarray(conv_b.reshape(L, 2, 128).transpose(2, 1, 0))
    fc1b_pack = np.ascontiguousarray(fc1_b.reshape(2, 128).T)

    in_maps = []
    for c in range(N_CORES):
        sl = slice(c * BC, (c + 1) * BC)
        in_maps.append({
            "seqT": np.ascontiguousarray(seq_var[sl].reshape(8, 128).T),
            "itemT": np.ascontiguousarray(item_var[sl].reshape(16, 128).T),
            "useri": np.ascontiguousarray(user_var[sl]),
            "item_emb": item_emb,
            "user_emb": user_emb,
            "w2tab": W2,
            "wt": wt_pack,
            "convb": convb_pack,
            "fc1wt": fc1wt,
            "fc1b": fc1b_pack,
        })
    return in_maps


def kernel(seq_var, user_var, item_var, item_emb, user_emb, conv_w, conv_b,
           fc1_w, fc1_b, W2, b2, _trace=False):
    from concourse import bass_utils
    nc = build_nc()
    in_maps = make_in_maps(seq_var, user_var, item_var, item_emb, user_emb,
                           conv_w, conv_b, fc1_w, fc1_b, W2, b2)
    r = bass_utils.run_bass_kernel_spmd(
        nc, in_maps, core_ids=list(range(N_CORES)), trace=_trace)
    out = np.concatenate([r.results[c]["res"] for c in range(N_CORES)], axis=0)
    b2 = np.asarray(b2, dtype=np.float32)
    item_var = np.asarray(item_var)
    out = out + b2[item_var][..., 0]
    if _trace:
        return out.astype(np.float32), r
    return out.astype(np.float32)


# revision 33
# speedup vs baseline: 1.0586x; 1.0586x over previous
"""Trainium2 Bass kernel for the QRNN-style recommender model.

Model (per batch row b):
  emb = item_emb[seq]                          # [T=16, D=256]
  conv_out[l,t,c] = sum_{m<=l} emb[t-m] @ W[l,m,c,:] + conv_b[l,c]   (L=16 causal convs)
  f = sigmoid(relu(conv_out)) = max(sigmoid(conv_out), 0.5)
  h = fo-pool chain applied 3x over t (QRNN), x0 = emb
  o = sum over (l, t) of h                     # [D]
  z = [o, user_emb[user]] @ fc1_w.T + fc1_b    # [D]
  res[n] = W2[item[n]] . z + b2[item[n]]       # [N_TGT=32]

Sharding: data-parallel over batch B=512 across 8 cores (64 rows each);
all parameters/tables replicated; embedding gathers via indirect DMA.

Per-core layout (v2 — unrolled fo-pool, no tensor_tensor_scan):
  embTb[kc][d(128), t(16), b(64)]  bf16 transposed gathered emb (conv rhs)
  X0b[d(128), t, cc(2), b]         fp16 emb for the fo-pool x0
  conv: per (l, cc) one psum tile [c(128), bank(2), t8, b64] (t-split
        banks, col = 64*t + b), accumulated over (m, kc) bf16 matmuls
  gates: one ACT sigmoid per (l, cc) -> F[d, t, cc, l8, b] fp16;
        DVE tensor_scalar max(., 0.5) clamp in place
  fo-pool: 2 rounds of 8 l's; per round 3 chains x 16 unrolled steps of
        fp16 DVE tensor_tensor (d = x - h; e = f*d; h = h + e), each op
        [128, 2*8*64] at DVE 2x rate; x of chain 0 broadcasts X0b over l
  o: in-place binary-tree adds over t then l, accumulated into oacc f32
  head: fc1 via PE, per-row dot with gathered W2 rows via DVE mul +
        ones-vector PE partition-reduction.
"""
import os
import numpy as np

import concourse.bass as bass
import concourse.mybir as mybir
import concourse.tile as tile
from concourse import bacc
from concourse.masks import make_identity

F32 = mybir.dt.float32
BF16 = mybir.dt.bfloat16
F16 = mybir.dt.float16
F8 = mybir.dt.float8e4
I32 = mybir.dt.int32
EMB_SCALE = 128.0          # emb fp8 pre-scale
W_SCALE = 32.0             # conv weight fp8 pre-scale
Z_DESCALE = 1.0 / (EMB_SCALE * W_SCALE)
AF = mybir.ActivationFunctionType
ALU = mybir.AluOpType

# model dims (hardcoded per problem spec)
N_CORES = 8
B = 512
BC = B // N_CORES          # 64 rows per core
T = 16
L = 16
L8 = 8                     # l's per round
D = 256
N_TGT = 32
N_ITEMS = 200000
N_USERS = 100000
N_L = 3                    # fo-pool chain depth
TRI = [l * (l + 1) // 2 for l in range(L + 1)]  # block offsets for (l, m<=l)


def _build_kernel(nc, tc):
    seqT = nc.dram_tensor("seqT", [128, 8], I32, kind="ExternalInput").ap()
    itemT = nc.dram_tensor("itemT", [128, 16], I32, kind="ExternalInput").ap()
    useri = nc.dram_tensor("useri", [BC], I32, kind="ExternalInput").ap()
    item_emb = nc.dram_tensor("item_emb", [N_ITEMS, D], F32, kind="ExternalInput").ap()
    user_emb = nc.dram_tensor("user_emb", [N_USERS, D], F32, kind="ExternalInput").ap()
    w2tab = nc.dram_tensor("w2tab", [N_ITEMS, D], F32, kind="ExternalInput").ap()
    wt = nc.dram_tensor("wt", [TRI[L], D, D], F8, kind="ExternalInput").ap()
    convb = nc.dram_tensor("convb", [128, 2, L], F32, kind="ExternalInput").ap()
    fc1wt = nc.dram_tensor("fc1wt", [2 * D, D], F32, kind="ExternalInput").ap()
    fc1b = nc.dram_tensor("fc1b", [128, 2], F32, kind="ExternalInput").ap()
    res = nc.dram_tensor("res", [BC, N_TGT], F32, kind="ExternalOutput").ap()

    import contextlib
    ctx = contextlib.ExitStack()
    with ctx:
        perm = ctx.enter_context(tc.tile_pool(name="perm", bufs=1))
        idxp = ctx.enter_context(tc.tile_pool(name="idxp", bufs=2))
        gath = ctx.enter_context(tc.tile_pool(name="gath", bufs=4))
        wpool = ctx.enter_context(tc.tile_pool(name="wpool", bufs=12))
        st = ctx.enter_context(tc.tile_pool(name="st", bufs=2))
        small = ctx.enter_context(tc.tile_pool(name="small", bufs=2))
        cps = ctx.enter_context(tc.tile_pool(name="cps", bufs=3, space="PSUM"))
        tps = ctx.enter_context(tc.tile_pool(name="tps", bufs=2, space="PSUM"))

        # all seq indices in one DMA so the 8 gathers issue back-to-back
        seqi = perm.tile([128, 8], I32, tag="seqi")
        nc.sync.dma_start(seqi[:], seqT[:])

        ident = perm.tile([128, 128], F32, tag="ident")
        make_identity(nc, ident)

        cb = perm.tile([128, 2, L], F32, tag="cb")
        nc.sync.dma_start(cb[:], convb[:])
        f1w = perm.tile([128, 4, D], F32, tag="f1w")
        nc.sync.dma_start(f1w[:], fc1wt.rearrange("(kc k) c -> k kc c", k=128))
        f1b = perm.tile([128, 2], F32, tag="f1b")
        nc.sync.dma_start(f1b[:], fc1b[:])

        # ---- phase A: gather seq embeddings -> embT8 (fp8, scaled), X0b (fp16)
        embT8 = perm.tile([128, 2, T, BC], F8, tag="embt8")
        X0b = perm.tile([128, T, 2, BC], F16, tag="x0b")
        for c in range(8):
            gt = gath.tile([128, D], F32, tag="embg")
            nc.gpsimd.indirect_dma_start(
                out=gt[:], out_offset=None, in_=item_emb[:],
                in_offset=bass.IndirectOffsetOnAxis(ap=seqi[:, c:c + 1], axis=0))
            for kc in (0, 1):
                tp = tps.tile([128, 128], F32, tag="tp")
                nc.tensor.transpose(tp[:], gt[:, kc * 128:(kc + 1) * 128], ident[:])
                tpv = tp[:].rearrange("p (b t) -> p t b", b=8)
                nc.scalar.mul(embT8[:, kc, :, 8 * c:8 * (c + 1)], tpv, EMB_SCALE)
                nc.vector.tensor_copy(out=X0b[:, :, kc, 8 * c:8 * (c + 1)], in_=tpv)

        # ---- big fp16 state tiles (round 0 split into two 4-l sub-rounds
        # so DVE steps start ~40us earlier; round 1 is one efficient 8-l round)
        F0a = perm.tile([128, T, 2, 4, BC], F16, tag="f0a")
        F0b = perm.tile([128, T, 2, 4, BC], F16, tag="f0b")
        F1 = perm.tile([128, T, 2, L8, BC], F16, tag="f1")
        HA = perm.tile([128, T, 2, L8, BC], F16, tag="ha")
        HB = perm.tile([128, T, 2, L8, BC], F16, tag="hb")
        oacc = [perm.tile([128, BC], F32, tag=f"oacc{cc}", name=f"oacc{cc}")
                for cc in (0, 1)]

        def conv_gates(l_lo, l_hi, Fti):
            """conv + sigmoid gates for l in [l_lo, l_hi) -> Fti."""
            DR = mybir.MatmulPerfMode.DoubleRow
            for gl in range(l_lo, l_hi):
                li = gl - l_lo
                m0max = min(gl, 7)
                # both cc psums live so each tap's weight is consumed immediately
                ps = [cps.tile([128, 2, 8, BC], F32, tag="cps", name=f"ps{cc}")
                      for cc in (0, 1)]
                for m in range(gl + 1):
                    w_t = wpool.tile([128, 2, D], F8, tag="wt")
                    nc.sync.dma_start(
                        w_t[:], wt[TRI[gl] + m].rearrange("(kc k) c -> k kc c", k=128))
                    for cc in (0, 1):
                        lhs = w_t[:, :, cc * 128:(cc + 1) * 128]
                        # bank 0: t in [0, 8)
                        if m <= 7:
                            nc.tensor.matmul(
                                ps[cc][:, 0, m:8, :], lhsT=lhs,
                                rhs=embT8[:, :, 0:8 - m, :],
                                start=(m == 0), stop=(m == m0max), perf_mode=DR)
                        # bank 1: t in [8, 16)
                        if m < 8:
                            out_ap = ps[cc][:, 1, :, :]
                            rhs = embT8[:, :, 8 - m:16 - m, :]
                        else:
                            out_ap = ps[cc][:, 1, m - 8:8, :]
                            rhs = embT8[:, :, 0:16 - m, :]
                        nc.tensor.matmul(
                            out_ap, lhsT=lhs, rhs=rhs,
                            start=(m == 0), stop=(m == gl), perf_mode=DR)
                for cc in (0, 1):
                    # f' = sigmoid(z); the relu fold (max with 0.5) happens as
                    # one whole-tile DVE clamp at the head of the fo-pool round
                    fsl = Fti[:, :, cc, li, :]
                    nc.scalar.activation(
                        fsl, ps[cc][:].rearrange("p bk t b -> p (bk t) b"),
                        AF.Sigmoid, bias=cb[:, cc, gl:gl + 1], scale=Z_DESCALE)

        def fopool(F, nl, rname):
            """triple fo-pool over t for gate tile F covering nl l's;
            returns O = sum over (t, l) as [128, 2, nl->1, BC]."""
            # f = sigmoid(relu(z)) = max(sigmoid(z), 0.5)
            nc.vector.tensor_scalar_max(out=F[:], in0=F[:], scalar1=0.5)
            ha = HA[:, :, :, 0:nl, :]
            hb = HB[:, :, :, 0:nl, :]
            # chains 1, 2 keep full h sequences (next chain's x)
            for xsrc, hout in ((None, ha), (ha, hb)):
                for t in range(T):
                    if xsrc is None:
                        xt = X0b[:, t, :, None, :].to_broadcast((128, 2, nl, BC))
                    else:
                        xt = xsrc[:, t]
                    ft = F[:, t]
                    if t == 0:
                        # h_0 = f_0 * x_0
                        nc.vector.tensor_tensor(
                            out=hout[:, 0], in0=ft, in1=xt, op=ALU.mult)
                    else:
                        # h_t = h_{t-1} + f_t * (x_t - h_{t-1})
                        d = st.tile([128, 2, nl, BC], F16, tag="std")
                        nc.vector.tensor_tensor(
                            out=d[:], in0=xt, in1=hout[:, t - 1], op=ALU.subtract)
                        e = st.tile([128, 2, nl, BC], F16, tag="ste")
                        nc.vector.tensor_tensor(
                            out=e[:], in0=ft, in1=d[:], op=ALU.mult)
                        nc.vector.tensor_tensor(
                            out=hout[:, t], in0=hout[:, t - 1], in1=e[:], op=ALU.add)
            # chain 3 with on-the-fly O accumulation
            O = perm.tile([128, 2, nl, BC], F16, tag=f"otile{rname}",
                          name=f"otile{rname}")
            h3p = None
            for t in range(T):
                xt = hb[:, t]
                ft = F[:, t]
                h3 = st.tile([128, 2, nl, BC], F16, tag="h3")
                if t == 0:
                    nc.vector.tensor_tensor(out=h3[:], in0=ft, in1=xt, op=ALU.mult)
                else:
                    d = st.tile([128, 2, nl, BC], F16, tag="std")
                    nc.vector.tensor_tensor(
                        out=d[:], in0=xt, in1=h3p[:], op=ALU.subtract)
                    e = st.tile([128, 2, nl, BC], F16, tag="ste")
                    nc.vector.tensor_tensor(out=e[:], in0=ft, in1=d[:], op=ALU.mult)
                    nc.vector.tensor_tensor(out=h3[:], in0=h3p[:], in1=e[:], op=ALU.add)
                    if t == 1:
                        nc.vector.tensor_tensor(out=O[:], in0=h3p[:], in1=h3[:], op=ALU.add)
                    else:
                        nc.vector.tensor_tensor(out=O[:], in0=O[:], in1=h3[:], op=ALU.add)
                h3p = h3
            # sum over l (in-place tree on O)
            wl = nl
            while wl > 1:
                wl //= 2
                nc.vector.tensor_tensor(
                    out=O[:, :, 0:wl], in0=O[:, :, 0:wl],
                    in1=O[:, :, wl:2 * wl], op=ALU.add)
            return O

        conv_gates(0, 4, F0a)
        with tc.tile_wait_until(1.0):
            Oa = fopool(F0a, 4, "a")
        conv_gates(4, 8, F0b)
        with tc.tile_wait_until(2.0):
            Ob = fopool(F0b, 4, "b")
        conv_gates(8, 16, F1)

        # ---- user embedding + W2 gathers/transposes: emitted here (unpinned)
        # so PE/ACT finish them during the fo-pool rounds, keeping the tail short
        uidx = idxp.tile([BC, 1], I32, tag="uidx")
        nc.sync.dma_start(uidx[:], useri[:, None])
        ug = gath.tile([BC, D], F32, tag="ug", bufs=1)
        nc.gpsimd.indirect_dma_start(
            out=ug[:], out_offset=None, in_=user_emb[:],
            in_offset=bass.IndirectOffsetOnAxis(ap=uidx[:, :1], axis=0))
        catT = [oacc[0], oacc[1]]
        for kc in (0, 1):
            tp = tps.tile([128, 128], F32, tag="tp")
            nc.tensor.transpose(tp[:, :BC], ug[:, kc * 128:(kc + 1) * 128], ident[:BC, :BC])
            ut = small.tile([128, BC], F32, tag=f"ut{kc}")
            nc.any.tensor_copy(ut[:], tp[:, :BC])
            catT.append(ut)

        # W2 row gathers -> w2t[kc] = [128, 2048] (c on partitions, (b,n) free)
        w2t = [perm.tile([128, BC * N_TGT], F32, tag=f"w2t{kc}", name=f"w2t{kc}")
               for kc in (0, 1)]
        itemi = perm.tile([128, 16], I32, tag="itemi")
        nc.sync.dma_start(itemi[:], itemT[:])
        for ch in range(16):
            wg = gath.tile([128, D], F32, tag="w2g")
            nc.gpsimd.indirect_dma_start(
                out=wg[:], out_offset=None, in_=w2tab[:],
                in_offset=bass.IndirectOffsetOnAxis(ap=itemi[:, ch:ch + 1], axis=0))
            for kc in (0, 1):
                tp = tps.tile([128, 128], F32, tag="tp")
                nc.tensor.transpose(tp[:], wg[:, kc * 128:(kc + 1) * 128], ident[:])
                nc.scalar.copy(w2t[kc][:, 128 * ch:128 * (ch + 1)], tp[:])

        with tc.tile_wait_until(2.5):
            Oc = fopool(F1, L8, "c")
        tail_pin = tc.tile_wait_until(3.0)
        tail_pin.__enter__()
        for cc in (0, 1):
            nc.scalar.copy(oacc[cc][:], Oa[:, cc, 0, :])
            for Ox in (Ob, Oc):
                stmp = small.tile([128, BC], F32, tag="stmp")
                nc.scalar.copy(stmp[:], Ox[:, cc, 0, :])
                nc.vector.tensor_tensor(
                    out=oacc[cc][:], in0=oacc[cc][:], in1=stmp[:], op=ALU.add)

        # ---- head: z^T = fc1_w @ cat^T + b  -> [zc(2 chunks of 128), b(64)]
        zT = []
        for cc in (0, 1):
            zp = tps.tile([128, BC], F32, tag="tp")
            for kcc in range(4):
                nc.tensor.matmul(
                    zp[:], lhsT=f1w[:, kcc, cc * 128:(cc + 1) * 128],
                    rhs=catT[kcc][:], start=(kcc == 0), stop=(kcc == 3))
            zt = small.tile([128, BC], F32, tag=f"zt{cc}")
            nc.scalar.activation(zt[:], zp[:], AF.Identity, bias=f1b[:, cc:cc + 1])
            zT.append(zt)

        # res[b,n] = sum_c w2t[c,(b,n)] * z[c,b]  (mul + ones-matmul partition sum)
        for kc in (0, 1):
            nc.vector.tensor_tensor(
                out=w2t[kc][:].rearrange("p (b n) -> p b n", n=N_TGT),
                in0=w2t[kc][:].rearrange("p (b n) -> p b n", n=N_TGT),
                in1=zT[kc][:, :, None].to_broadcast((128, BC, N_TGT)),
                op=ALU.mult)
        ones = small.tile([128, 1], F32, tag="ones")
        nc.vector.memset(ones[:], 1.0)
        res_sb = small.tile([1, BC * N_TGT], F32, tag="ressb", bufs=1)
        for j in range(4):
            rj = tps.tile([1, 512], F32, tag="tp")
            for kc in (0, 1):
                nc.tensor.matmul(rj[:], lhsT=ones[:],
                                 rhs=w2t[kc][:, 512 * j:512 * (j + 1)],
                                 start=(kc == 0), stop=(kc == 1))
            nc.any.tensor_copy(res_sb[:, 512 * j:512 * (j + 1)], rj[:])
        nc.sync.dma_start(res.rearrange("b n -> (b n)")[None, :], res_sb[:])
        tail_pin.__exit__(None, None, None)


_CACHED_NC = None


def build_nc():
    global _CACHED_NC
    if _CACHED_NC is not None:
        return _CACHED_NC
    nc = bacc.Bacc("TRN2", debug=False, enable_asserts=False)
    with tile.TileContext(nc) as tc:
        _build_kernel(nc, tc)
    nc.compile()
    _CACHED_NC = nc
    return nc


def make_in_maps(seq_var, user_var, item_var, item_emb, user_emb, conv_w,
                 conv_b, fc1_w, fc1_b, W2, b2):
    seq_var = np.asarray(seq_var).astype(np.int32)
    user_var = np.asarray(user_var).astype(np.int32)
    item_var = np.asarray(item_var).astype(np.int32)
    item_emb = np.ascontiguousarray(np.asarray(item_emb, dtype=np.float32))
    user_emb = np.ascontiguousarray(np.asarray(user_emb, dtype=np.float32))
    W2 = np.ascontiguousarray(np.asarray(W2, dtype=np.float32))
    conv_w = np.asarray(conv_w, dtype=np.float32)
    conv_b = np.ascontiguousarray(np.asarray(conv_b, dtype=np.float32))
    fc1_w = np.asarray(fc1_w, dtype=np.float32)
    fc1_b = np.ascontiguousarray(np.asarray(fc1_b, dtype=np.float32))

    # pack conv weights: block (l, m<=l) at TRI[l]+m = conv_w[l, m].T ([d, c]),
    # fp8 e4m3 scaled by W_SCALE (emb scaled by EMB_SCALE; ACT rescales psum)
    import ml_dtypes
    wt_pack = np.empty((TRI[L], D, D), ml_dtypes.float8_e4m3)
    for l in range(L):
        for m in range(l + 1):
            wt_pack[TRI[l] + m] = (conv_w[l, m].T * W_SCALE).astype(
                ml_dtypes.float8_e4m3)
    fc1wt = np.ascontiguousarray(fc1_w.T)
    # convb_pack[c, cc, l] = conv_b[l, cc*128 + c];  fc1b_pack[c, cc] = fc1_b[cc*128+c]
    convb_pack = np.ascontiguousarray(conv_b.reshape(L, 2, 128).transpose(2, 1, 0))
    fc1b_pack = np.ascontiguousarray(fc1_b.reshape(2, 128).T)

    in_maps = []
    for c in range(N_CORES):
        sl = slice(c * BC, (c + 1) * BC)
        in_maps.append({
            "seqT": np.ascontiguousarray(seq_var[sl].reshape(8, 128).T),
            "itemT": np.ascontiguousarray(item_var[sl].reshape(16, 128).T),
            "useri": np.ascontiguousarray(user_var[sl]),
            "item_emb": item_emb,
            "user_emb": user_emb,
            "w2tab": W2,
            "wt": wt_pack,
            "convb": convb_pack,
            "fc1wt": fc1wt,
            "fc1b": fc1b_pack,
        })
    return in_maps


def kernel(seq_var, user_var, item_var, item_emb, user_emb, conv_w, conv_b,
           fc1_w, fc1_b, W2, b2, _trace=False):
    from concourse import bass_utils
    nc = build_nc()
    in_maps = make_in_maps(seq_var, user_var, item_var, item_emb, user_emb,
                           conv_w, conv_b, fc1_w, fc1_b, W2, b2)
    r = bass_utils.run_bass_kernel_spmd(
        nc, in_maps, core_ids=list(range(N_CORES)), trace=_trace)
    out = np.concatenate([r.results[c]["res"] for c in range(N_CORES)], axis=0)
    b2 = np.asarray(b2, dtype=np.float32)
    item_var = np.asarray(item_var)
    out = out + b2[item_var][..., 0]
    if _trace:
        return out.astype(np.float32), r
    return out.astype(np.float32)
